# revision 12
# baseline (speedup 1.0000x reference)
"""2-layer GCN on 8 Trainium2 NeuronCores (Bass/Tile), self-contained.

Sharding: nodes partitioned across 8 cores (12500 rows each), weights
replicated. Per core, per layer: compute its table shard (x @ W) * dinv in
bf16, AllGather the compact [N,64] table, then aggregate incoming edge
messages with dma_gather (256B padded rows) + one-hot segment matmuls.

v3 vs v1: bf16 tables/messages/matmuls/one-hots (half HBM bytes, 4x PE),
int32->int16 window regions aligned to one AllGather, 80 large gather
calls per layer instead of 392 small ones, idx loaded once as one const,
host-precomputed xT/dinv (no device transposes in table build), fused
epilogues, AllGather writes strided into the padded gather table.
"""
import math

import numpy as np

N = 100000
E_RAW = 1600000
DIN = 64
DH = 64
DOUT = 16
NCORE = 8
SH = 12500            # nodes per core
T = 98                # dst tiles per core (128 nodes each)
SHP = 128 * T         # padded shard rows = 12544
NTAB = NCORE * SHP    # gather table rows = 100352
RSIZE = 32768         # int16 index window (rows)
NR = 4                # ceil(NTAB / RSIZE) regions
SG = 5                # tiles per gather call-group
PADP = 999.0          # dl marker for padded edges (kills one-hot col)

_CACHE = {}


def _host_prep(edge_index):
    """Build per-core gather/one-hot metadata. Returns (meta, percore)."""
    src = np.concatenate([edge_index[0], np.arange(N, dtype=np.int64)])
    dst = np.concatenate([edge_index[1], np.arange(N, dtype=np.int64)])
    src = src.astype(np.int64)
    dst = dst.astype(np.int64)

    deg = np.bincount(dst, minlength=N).astype(np.float64)
    dinv_g = (1.0 / np.sqrt(deg)).astype(np.float32)   # deg >= 1 (self loops)

    k = dst // SH                       # owning core of each edge (by dst)
    jj = dst - k * SH                   # local dst node
    t_dst = jj // 128                   # dst tile
    p_dst = jj % 128                    # psum slot
    trow = (src // SH) * SHP + (src % SH)    # gather-table row of src
    r = trow // RSIZE                   # int16 window region
    loc = trow % RSIZE

    gid = ((k * T + t_dst) * NR + r).astype(np.int64)
    order = np.argsort(gid, kind="stable")
    gid_s = gid[order]
    loc_s = loc[order].astype(np.int32)
    p_s = p_dst[order].astype(np.float32)

    ngroups = NCORE * T * NR
    counts = np.bincount(gid_s, minlength=ngroups).reshape(NCORE, T, NR)
    starts = np.zeros(ngroups + 1, np.int64)
    np.cumsum(counts.reshape(-1), out=starts[1:])

    n_chunk = -(-counts.max(axis=0) // 128)            # [T, NR]
    sbs = [list(range(s, min(s + SG, T))) for s in range(0, T, SG)]

    meta = dict(n_chunk=n_chunk, sbs=sbs)
    tot_cols = int(n_chunk.sum())
    meta["tot_cols"] = tot_cols

    import ml_dtypes
    bf = ml_dtypes.bfloat16

    percore = []
    for kk in range(NCORE):
        # idx in call order (g, r, t, c); dl in mm order (g, t, r, c)
        idx_parts = []
        dl_cols = []

        def group_arrays(tt, rr):
            g = (kk * T + tt) * NR + rr
            cnt = counts[kk, tt, rr]
            npad = n_chunk[tt, rr] * 128
            lo = np.zeros(npad, np.int32)
            pp = np.full(npad, PADP, np.float32)
            lo[:cnt] = loc_s[starts[g]:starts[g] + cnt]
            pp[:cnt] = p_s[starts[g]:starts[g] + cnt]
            return lo, pp

        for tiles in sbs:
            for rr in range(NR):
                for tt in tiles:
                    idx_parts.append(group_arrays(tt, rr)[0])
            for tt in tiles:
                for rr in range(NR):
                    pp = group_arrays(tt, rr)[1]
                    if pp.size:
                        dl_cols.append(pp.reshape(-1, 128).T)  # [128, nchunk]
        flat = np.concatenate(idx_parts).astype(np.int16)
        ix = np.tile(flat.reshape(-1, 16).T, (8, 1)).copy()     # [128, n/16]
        dl = np.concatenate(dl_cols, axis=1).astype(np.float32)  # [128, totc]

        dv = np.zeros(SHP, np.float32)
        dv[:SH] = dinv_g[kk * SH:(kk + 1) * SH]
        dinv_pm = np.ascontiguousarray(dv.reshape(T, 128).T)    # [128, T]
        percore.append(dict(ix=ix, dl=dl, dinv=dinv_pm))
    meta["idx_len"] = percore[0]["ix"].shape[1]
    return meta, percore


def _build_nc(meta, npass=1, msg_bufs=8, oh_bufs=6, mode="full",
              strided_ag=False, act_mod=0, ps_bufs=3):
    import concourse.bacc as bacc
    import concourse.mybir as mybir
    from concourse.masks import make_identity
    from concourse.tile import TileContext

    f32 = mybir.dt.float32
    bf16 = mybir.dt.bfloat16
    n_chunk = meta["n_chunk"]
    sbs = meta["sbs"]
    tot_cols = meta["tot_cols"]
    idx_len = meta["idx_len"]

    nc = bacc.Bacc("TRN2", target_bir_lowering=False, debug=False,
                   num_devices=NCORE, num_swdge_queues=4)
    xT_d = nc.dram_tensor("xT", [DIN, SHP], bf16, kind="ExternalInput")
    dinv_d = nc.dram_tensor("dinv", [128, T], f32, kind="ExternalInput")
    w1_d = nc.dram_tensor("w1", [DIN, DH], bf16, kind="ExternalInput")
    w2_d = nc.dram_tensor("w2", [DH, DOUT], bf16, kind="ExternalInput")
    b1_d = nc.dram_tensor("b1b", [128, DH], f32, kind="ExternalInput")
    b2_d = nc.dram_tensor("b2b", [128, DOUT], f32, kind="ExternalInput")
    io_d = nc.dram_tensor("iota2d", [128, 128], bf16, kind="ExternalInput")
    dl_d = nc.dram_tensor("dl", [128, tot_cols], f32, kind="ExternalInput")
    ix_d = nc.dram_tensor("ix", [128, idx_len], mybir.dt.int16,
                          kind="ExternalInput")
    y_d = nc.dram_tensor("y_pm", [128, T * DOUT], f32, kind="ExternalOutput")

    qrot = [0]

    def nextq():
        qrot[0] = (qrot[0] + 1) % 4
        return qrot[0]

    with TileContext(nc) as tc:
        with (
            tc.tile_pool(name="const", bufs=1) as constp,
            tc.tile_pool(name="big", bufs=2) as bigp,
            tc.tile_pool(name="msg", bufs=msg_bufs) as msgp,
            tc.tile_pool(name="ohp", bufs=oh_bufs) as ohp,
            tc.tile_pool(name="work", bufs=3) as workp,
            tc.tile_pool(name="ps", bufs=ps_bufs, space="PSUM") as psp,
            tc.tile_pool(name="ps2", bufs=2, space="PSUM") as ps2p,
            tc.tile_pool(name="dram", bufs=2, space="DRAM") as dramp,
        ):
            ident = constp.tile([128, 128], bf16)
            make_identity(nc, ident[:])
            w1_s = constp.tile([DIN, DH], bf16)
            nc.sync.dma_start(out=w1_s[:], in_=w1_d[:])
            w2_s = constp.tile([DH, DOUT], bf16)
            nc.sync.dma_start(out=w2_s[:], in_=w2_d[:])
            b1_s = constp.tile([128, DH], f32)
            nc.sync.dma_start(out=b1_s[:], in_=b1_d[:])
            b2_s = constp.tile([128, DOUT], f32)
            nc.sync.dma_start(out=b2_s[:], in_=b2_d[:])
            iota_s = constp.tile([128, 128], bf16)
            nc.sync.dma_start(out=iota_s[:], in_=io_d[:])
            dl_s = constp.tile([128, tot_cols], f32)
            nc.sync.dma_start(out=dl_s[:], in_=dl_d[:])
            ix_s = constp.tile([128, idx_len], mybir.dt.int16)
            nc.sync.dma_start(out=ix_s[:], in_=ix_d[:])
            dinv_s = constp.tile([128, T], f32)
            nc.sync.dma_start(out=dinv_s[:], in_=dinv_d[:])
            xT_s = constp.tile([DIN, SHP], bf16)
            nc.sync.dma_start(out=xT_s[:], in_=xT_d[:])

            def one_pass():
                # ---- table1 = (x @ W1) * dinv ----
                tab1 = bigp.tile([128, T * DH], bf16, tag="tab1")
                for t in range(T):
                    h_ps = psp.tile([128, DH], f32, tag="agg", name="h_ps")
                    nc.tensor.matmul(h_ps[:],
                                     lhsT=xT_s[:, t * 128:(t + 1) * 128],
                                     rhs=w1_s[:], start=True, stop=True)
                    nc.vector.tensor_scalar(
                        out=tab1[:, t * DH:(t + 1) * DH], in0=h_ps[:],
                        scalar1=dinv_s[:, t:t + 1], scalar2=None,
                        op0=mybir.AluOpType.mult)

                def allgather(tab_sb, tag):
                    ag_in = dramp.tile([SHP, DH], bf16, tag=f"agin{tag}")
                    nc.sync.dma_start(
                        out=ag_in[:].rearrange("(t p) f -> p t f", p=128),
                        in_=tab_sb[:].rearrange("p (t f) -> p t f", f=DH))
                    full = dramp.tile([NTAB, DH], bf16, tag=f"full{tag}",
                                      addr_space="Shared")
                    nc.gpsimd.collective_compute(
                        "AllGather", mybir.AluOpType.bypass,
                        replica_groups=[list(range(NCORE))],
                        ins=[ag_in[:]], outs=[full[:]])
                    pads = []
                    for rr in range(NR):
                        rlen = min(RSIZE, NTAB - rr * RSIZE)
                        pr = dramp.tile([rlen, 128], bf16,
                                        tag=f"pad{tag}r{rr}")
                        nc.sync.dma_start(
                            out=pr[:, 0:DH],
                            in_=full[rr * RSIZE:rr * RSIZE + rlen, :])
                        pads.append(pr)
                    return pads

                tab1_pads = allgather(tab1, "1")

                def aggregate(pads, epilogue):
                    """Gather + one-hot segment matmuls; epilogue(t, psum)."""
                    ch_col = 0          # global dl column
                    ix_off = 0          # int16 wrapped-col offset into ix_s
                    for tiles in sbs:
                        msgs = {}
                        for rr in range(NR):
                            cols = int(sum(n_chunk[tt, rr] for tt in tiles))
                            if cols == 0 or mode == "none":
                                msgs[rr] = None
                                continue
                            m = msgp.tile([128, cols, 128], bf16, tag="m")
                            nidx = cols * 128
                            nc.gpsimd.dma_gather(
                                out_ap=m[:],
                                in_ap=pads[rr][:],
                                idxs_ap=ix_s[:, ix_off:ix_off + nidx // 16],
                                num_idxs=nidx,
                                num_idxs_reg=nidx,
                                elem_size=128,
                                queue_num=nextq(),
                                single_packet=False,
                            )
                            ix_off += nidx // 16
                            msgs[rr] = m
                        for ti, tt in enumerate(tiles):
                            nch = int(sum(n_chunk[tt, rr] for rr in range(NR)))
                            if mode == "full" and nch > 0:
                                ps = psp.tile([128, DH], f32, tag="agg",
                                              name="ps_agg")
                            else:
                                ps = None
                            done = 0
                            for rr in range(NR):
                                base = int(sum(n_chunk[t2, rr]
                                               for t2 in tiles[:ti]))
                                for c in range(int(n_chunk[tt, rr])):
                                    if mode == "full":
                                        oh = ohp.tile([128, 128], bf16,
                                                      tag="oh")
                                        if act_mod and ch_col % act_mod == 0:
                                            sq = ohp.tile([128, 128], bf16,
                                                          tag="sq")
                                            nc.scalar.activation(
                                                sq[:], iota_s[:],
                                                mybir.ActivationFunctionType.Square,
                                                bias=dl_s[:, ch_col:ch_col + 1],
                                                scale=-1.0)
                                            nc.scalar.activation(
                                                oh[:], sq[:],
                                                mybir.ActivationFunctionType.Relu,
                                                bias=1.0, scale=-1.0)
                                        else:
                                            nc.vector.tensor_scalar(
                                                out=oh[:], in0=iota_s[:],
                                                scalar1=dl_s[:, ch_col:ch_col + 1],
                                                scalar2=None,
                                                op0=mybir.AluOpType.is_equal)
                                        nc.tensor.matmul(
                                            ps[:], lhsT=oh[:],
                                            rhs=msgs[rr][:, base + c, 0:DH],
                                            start=(done == 0),
                                            stop=(done == nch - 1))
                                    ch_col += 1
                                    done += 1
                            epilogue(tt, ps)

                # ---- layer 1 ----
                tab2 = bigp.tile([128, T * DH], bf16, tag="tab2")

                def epi1(tt, ps):
                    src = ps[:] if ps is not None else b1_s[:]
                    u = workp.tile([128, DH], f32, tag="u")
                    nc.vector.tensor_scalar(
                        out=u[:], in0=src, scalar1=dinv_s[:, tt:tt + 1],
                        scalar2=None, op0=mybir.AluOpType.mult)
                    v = workp.tile([128, DH], f32, tag="v")
                    nc.vector.tensor_tensor(out=v[:], in0=u[:], in1=b1_s[:],
                                            op=mybir.AluOpType.add)
                    nc.vector.tensor_scalar(
                        out=tab2[:, tt * DH:(tt + 1) * DH], in0=v[:],
                        scalar1=dinv_s[:, tt:tt + 1], scalar2=0.0,
                        op0=mybir.AluOpType.mult, op1=mybir.AluOpType.max)

                aggregate(tab1_pads, epi1)
                tab2_pads = allgather(tab2, "2")

                # ---- layer 2 ----
                ybuf = bigp.tile([128, T * DOUT], f32, tag="ybuf")

                def epi2(tt, ps):
                    src = ps[:] if ps is not None else b1_s[:]
                    s1 = workp.tile([128, DH], bf16, tag="s1")
                    nc.vector.tensor_scalar(
                        out=s1[:], in0=src, scalar1=dinv_s[:, tt:tt + 1],
                        scalar2=None, op0=mybir.AluOpType.mult)
                    tr_ps = ps2p.tile([DH, 128], bf16, tag="tr")
                    nc.tensor.transpose(out=tr_ps[:], in_=s1[:],
                                        identity=ident[:])
                    sT = workp.tile([DH, 128], bf16, tag="sT")
                    nc.vector.tensor_copy(out=sT[:], in_=tr_ps[:])
                    o_ps = ps2p.tile([128, DOUT], f32, tag="o")
                    nc.tensor.matmul(o_ps[:], lhsT=sT[:], rhs=w2_s[:],
                                     start=True, stop=True)
                    o1 = workp.tile([128, DOUT], f32, tag="o1")
                    nc.vector.tensor_tensor(out=o1[:], in0=o_ps[:],
                                            in1=b2_s[:],
                                            op=mybir.AluOpType.add)
                    nc.scalar.activation(ybuf[:, tt * DOUT:(tt + 1) * DOUT],
                                         o1[:],
                                         mybir.ActivationFunctionType.Sigmoid)

                aggregate(tab2_pads, epi2)
                nc.sync.dma_start(out=y_d[:], in_=ybuf[:])

            for _pass in range(npass):
                one_pass()

    nc.compile()
    return nc


def _make_runner(nc, n_cores):
    import jax
    from jax.sharding import Mesh, NamedSharding, PartitionSpec
    from jax.experimental.shard_map import shard_map
    import concourse.mybir as mybir
    from concourse import bass2jax

    bass2jax.install_neuronx_cc_hook()
    partition_name = (nc.partition_id_tensor.name
                      if nc.partition_id_tensor else None)
    in_names, out_names, out_avals, zero_outs = [], [], [], []
    for alloc in nc.m.functions[0].allocations:
        if not isinstance(alloc, mybir.MemoryLocationSet):
            continue
        name = alloc.memorylocations[0].name
        if alloc.kind == "ExternalInput":
            if name != partition_name:
                in_names.append(name)
        elif alloc.kind == "ExternalOutput":
            out_names.append(name)
            shape = tuple(alloc.tensor_shape)
            dtype = mybir.dt.np(alloc.dtype)
            out_avals.append(jax.core.ShapedArray(shape, dtype))
            zero_outs.append(np.zeros(shape, dtype))
    n_params = len(in_names)
    all_in = list(in_names) + list(out_names)
    if partition_name is not None:
        all_in.append(partition_name)

    def _body(*args):
        operands = list(args)
        if partition_name is not None:
            operands.append(bass2jax.partition_id_tensor())
        outs = bass2jax._bass_exec_p.bind(
            *operands, out_avals=tuple(out_avals), in_names=tuple(all_in),
            out_names=tuple(out_names), lowering_input_output_aliases=(),
            sim_require_finite=True, sim_require_nnan=True, nc=nc)
        return tuple(outs)

    devices = jax.devices()[:n_cores]
    mesh = Mesh(np.asarray(devices), ("core",))
    nspec = (PartitionSpec("core"),)
    sharded = jax.jit(
        shard_map(_body, mesh=mesh, in_specs=nspec * (n_params + len(out_names)),
                  out_specs=nspec * len(out_names), check_rep=False),
        keep_unused=True)
    sh = NamedSharding(mesh, PartitionSpec("core"))

    def place(in_maps):
        per_core = [[np.asarray(m[nm]) for nm in in_names] for m in in_maps]
        concat = [np.concatenate([per_core[c][i] for c in range(n_cores)], 0)
                  for i in range(n_params)]
        concat += [np.zeros((n_cores * z.shape[0], *z.shape[1:]), z.dtype)
                   for z in zero_outs]
        placed = [jax.device_put(a, sh) for a in concat]
        jax.block_until_ready(placed)
        return placed

    def run(placed):
        out = sharded(*placed)
        jax.block_until_ready(out)
        return out

    return place, run, out_names, out_avals


def _get_compiled(edge_index_key, edge_index):
    if edge_index_key in _CACHE:
        return _CACHE[edge_index_key]
    meta, percore = _host_prep(edge_index)
    nc = _build_nc(meta)
    place, run, out_names, out_avals = _make_runner(nc, NCORE)
    _CACHE[edge_index_key] = (meta, percore, place, run, out_names, out_avals)
    return _CACHE[edge_index_key]


def _in_maps(percore, x, W1, b1, W2, b2):
    import ml_dtypes
    bf = ml_dtypes.bfloat16
    x = np.asarray(x, np.float32)
    maps = []
    iota = np.tile(np.arange(128, dtype=np.float32)[None, :],
                   (128, 1)).astype(bf)
    w1 = np.asarray(W1, np.float32).astype(bf)
    w2 = np.asarray(W2, np.float32).astype(bf)
    b1b = np.tile(np.asarray(b1, np.float32)[None, :], (128, 1))
    b2b = np.tile(np.asarray(b2, np.float32)[None, :], (128, 1))
    for kk in range(NCORE):
        xs = np.zeros((SHP, DIN), np.float32)
        xs[:SH] = x[kk * SH:(kk + 1) * SH]
        pc = percore[kk]
        m = {
            "xT": np.ascontiguousarray(xs.T).astype(bf),
            "dinv": pc["dinv"],
            "w1": w1, "w2": w2, "b1b": b1b, "b2b": b2b,
            "iota2d": iota,
            "dl": pc["dl"],
            "ix": pc["ix"],
        }
        maps.append(m)
    return maps


def kernel(x, edge_index, W1, b1, W2, b2):
    ei = np.asarray(edge_index)
    key = hash(ei.tobytes())
    meta, percore, place, run, out_names, out_avals = _get_compiled(key, ei)
    maps = _in_maps(percore, x, W1, b1, W2, b2)
    placed = place(maps)
    out = run(placed)
    yi = out_names.index("y_pm")
    y_all = np.asarray(out[yi]).reshape(NCORE, 128, T * DOUT)
    res = np.empty((N, DOUT), np.float32)
    for kk in range(NCORE):
        shard = y_all[kk].reshape(128, T, DOUT).transpose(1, 0, 2)
        res[kk * SH:(kk + 1) * SH] = shard.reshape(SHP, DOUT)[:SH]
    return res
